# revision 6
# baseline (speedup 1.0000x reference)
"""Contrastive loss kernel — symmetric-half computation on 8 TRN2 cores.

Math (reference):
    s = cosine similarity matrix of x [8192, 256], tau = 0.1
    d_i = sum_j exp(s_ij * m_ij / tau)   (diag masked pre-exp -> contributes 1)
    v_i = s[i, i^1]
    loss = mean(log d_i - v_i / tau)

Key idea vs v1: S is symmetric, so each exp'd block can serve BOTH the row
sums of its rows (ACT accum / DVE reduce along free axis) AND, via a cheap
ones-weight matmul that contracts the partition axis, the row sums of the
transposed rows (exported as "column sum" partials, combined on host).
Each core computes ~56% of its v1 share:
  - main rect: its 1024 rows x local cols [0, 4096)  (wrapped: own band
    + next 3 bands)
  - d=4 band split in quadrants so the program is identical on all cores:
    m-tiles 0-3 x local cols [4096, 4608) and m-tiles 4-7 x [4608, 5120);
    the host maps those local cols to different global halves per core.
Row sums for cols [0,1024) (own band, computed as a full square) need no
colsum; cols >= 1024 export colsum partials.

Positive-pair logits v_i and the diagonal exp(s_ii/tau) are recomputed on
the host from the SAME quantized inputs the device uses (exact to fp32
rounding + ACT-spline ULPs) — this removes the v1 mask/extraction work from
DVE entirely.

fp8 path: inputs are scaled by 64 and quantized to fp8e4 (TRN e4m3); the
matmuls use DoubleRow perf mode (K=256 contracted in ONE pass, 2 fp8
MACs/cell/cycle); exp outputs for colsum-bearing supertiles are written as
fp8 PAIRS [128, 2, width] so the colsum matmuls also run in DoubleRow
(one matmul reduces 256 rows). The own-band supertile's exp output stays
bf16 (diag holds e^10 ~ 22026 > fp8 max).

Row sums are split between ACT (fused accum_out, +~280ns/instr accumulator
read) and DVE (reduce_sum from SBUF, ~1.1us per 1024-wide tile) to balance
the two engines; KERNEL_V2_ACC tunes the split.
"""

import os
import sys

import numpy as np

sys.path.insert(0, "/opt/trn_rl_repo")

import concourse.bass as bass
import concourse.tile as tile
from concourse import mybir
from concourse.bass_utils import run_bass_kernel_spmd

TAU = 0.1
N = 8192
D = 256
P = 128
NCORES = 8
RPC = N // NCORES            # 1024 rows per core
M_TILES = RPC // P           # 8
LCOLS = 5120                 # local cols: 4096 main + 2x512 extra
MAIN = 4096
SUPER = 1024                 # main supertile width (2 PSUM banks fp32)
NS = 5                       # s=0..3 main supertiles, s=4 extra
N_CS = 9                     # colsum chunks of 512 (6 main + 2 extra + tri)
TRI = os.environ.get("KERNEL_V2_TRI", "1") == "1"   # own-band triangle trim
SUPER_W = int(os.environ.get("KERNEL_V2_SUPER", "1024"))
if SUPER_W == 1536:
    MAIN_GROUPS = [(1024, 1536), (2560, 1536)]
else:
    MAIN_GROUPS = [(1024, 1024), (2048, 1024), (3072, 1024)]
N_GROUPS = 2 + len(MAIN_GROUPS)      # s0 + extra + main groups
N_ACC = N_GROUPS * M_TILES
PS_W = max(SUPER_W, 1024)            # psum tile width
PS_BUFS = 2 if SUPER_W == 1536 else 3

FP32 = mybir.dt.float32
BF16 = mybir.dt.bfloat16
FP8 = mybir.dt.float8e4

MM_DT = os.environ.get("KERNEL_V2_DT", "fp8")     # "fp8" | "bf16"
SCALE_IN = 64.0 if MM_DT == "fp8" else 1.0
ACT_SCALE = 1.0 / (TAU * SCALE_IN * SCALE_IN)
# which ACT instrs use fused accum_out for the row sum (rest use DVE
# reduce): "all" | "none" | "K" = every K-th instr (spread, so DVE's
# slightly-slower reduces never fall behind ACT) — last 2 always accum
# so no DVE reduce trails the final ACT.
ACC_SPEC = os.environ.get("KERNEL_V2_ACC", "4")
N_ACT_TOT = N_GROUPS * M_TILES

_CACHE = {}


def _use_accum(n_emitted):
    if ACC_SPEC == "all":
        return True
    if ACC_SPEC == "none":
        return False
    if n_emitted >= N_ACT_TOT - 2:
        return True
    return n_emitted % int(ACC_SPEC) == 0


def build_nc(repeat=1):
    mmdt = FP8 if MM_DT == "fp8" else BF16
    eodt = FP8 if MM_DT == "fp8" else BF16   # colsum-bearing exp outputs
    nc = bass.Bass(trn_type="TRN2")
    xm_d = nc.declare_dram_parameter("xm", [2, P, LCOLS], mmdt, isOutput=False)
    ones_d = nc.declare_dram_parameter("ones", [P, 2, 16], eodt, isOutput=False)
    acc_d = nc.declare_dram_parameter("acc", [P, N_ACC], FP32, isOutput=True)
    cs_d = nc.declare_dram_parameter("cs", [1, N_CS * 512], FP32, isOutput=True)

    import contextlib
    with tile.TileContext(nc) as tc:
        with (
            tc.tile_pool(name="big", bufs=1) as big,
            tc.tile_pool(name="small", bufs=1) as small,
            tc.tile_pool(name="eo0p", bufs=3) as eo0p,
            tc.tile_pool(name="eopp", bufs=4) as eopp,
            tc.tile_pool(name="psum", bufs=int(os.environ.get("KERNEL_V2_PPB", str(PS_BUFS))), space="PSUM") as pp,
            tc.tile_pool(name="pscs", bufs=int(os.environ.get("KERNEL_V2_CSB", "2")), space="PSUM") as ppc,
        ):
            xm = big.tile([P, 2, LCOLS], mmdt, tag="xm")
            ones_t = small.tile([P, 2, 16], eodt, tag="ones")
            acc_sb = small.tile([P, N_ACC], FP32, tag="accsb")
            cs_sb = small.tile([1, N_CS * 512], FP32, tag="cssb")

            nc.sync.dma_start(out=ones_t, in_=ones_d[:, :, :])

            # Warmup: load the Exp table + accum path on ACT, a few matmuls
            # to start HAM warm-up, DVE touch.  All from memset tiles so the
            # only DMA dependency is ones_t.
            warm_bf = small.tile([P, P], BF16, tag="warm_bf")
            nc.vector.memset(warm_bf, 1.0)
            warm_in = small.tile([P, 16], FP32, tag="warm_in")
            nc.vector.memset(warm_in, 0.25)
            warm_a = small.tile([P, 16], FP32, tag="warm_a")
            warm_acc = small.tile([P, 1], FP32, tag="warm_acc")
            nc.scalar.activation(out=warm_a, in_=warm_in,
                                 func=mybir.ActivationFunctionType.Exp,
                                 scale=1.0, accum_out=warm_acc)
            ps_w = pp.tile([P, PS_W], FP32, tag="super")
            for _ in range(6):
                nc.tensor.matmul(ps_w[:, 0:P], warm_bf, warm_bf,
                                 start=True, stop=True)
            # make ACT observe the ones DMA so later waits stay single
            warm_o = small.tile([P, 2], FP32, tag="warm_o")
            nc.scalar.copy(out=warm_o, in_=ones_t[:, :, 0])

            loop_ctx = (tc.For_i(0, repeat, 1)
                        if repeat > 1 else contextlib.nullcontext())
            with loop_ctx:
                _compute_body(nc, tc, big, small, eo0p, eopp, pp, ppc,
                              xm, xm_d, ones_t, warm_bf, acc_sb, cs_sb)

            nc.sync.dma_start(out=acc_d[:, :], in_=acc_sb)
            nc.sync.dma_start(out=cs_d[:, :], in_=cs_sb)
    _split_multi_waits(nc)
    return nc


def _compute_body(nc, tc, big, small, eo0p, eopp, pp, ppc,
                  xm, xm_d, ones_t, ones_bf, acc_sb, cs_sb):
    fp8 = MM_DT == "fp8"
    DR = mybir.MatmulPerfMode.DoubleRow
    pe_only = os.environ.get("KERNEL_V2_PE_ONLY", "0") == "1"
    no_cs = os.environ.get("KERNEL_V2_NO_CS", "0") == "1"

    # chunked input DMA: 1024-col chunks per k-half in PROCESS order (the
    # weights region [0,1024) doubles as s0's rhs); first chunk split in
    # 512s so the first supertile's operands arrive ~0.5us earlier
    for piece in range(2):
        cs_ = slice(piece * 512, (piece + 1) * 512)
        nc.sync.dma_start(out=xm[:, 0, cs_], in_=xm_d[0, :, cs_])
        nc.sync.dma_start(out=xm[:, 1, cs_], in_=xm_d[1, :, cs_])
    for a_, b_ in [(4096, 5120), (1024, 3072), (3072, 4096)]:
        cs_ = slice(a_, b_)   # process order: extra chunk, then main
        nc.sync.dma_start(out=xm[:, 0, cs_], in_=xm_d[0, :, cs_])
        nc.sync.dma_start(out=xm[:, 1, cs_], in_=xm_d[1, :, cs_])

    if pe_only or no_cs:
        nc.vector.memset(acc_sb, 1.0)
        nc.vector.memset(cs_sb, 0.0)

    n_act = 0          # ACT instrs emitted (for the accum-split knob)
    pending = []       # delayed colsum ops: emitted after LATER exp-mms
    DELAY = int(os.environ.get("KERNEL_V2_CSDELAY", "1"))

    def flush_pending(all_=False):
        while pending and (all_ or len(pending) > DELAY):
            pending.pop(0)()

    # process order: own band (its DMA chunk doubles as the weights region),
    # then the d=4 extra, then the main band groups
    groups = [("s0", 0, 1024), ("x", MAIN, 512)] + \
        [("main", c0g, wg) for c0g, wg in MAIN_GROUPS]
    for gi, (kind, gc0, gw) in enumerate(groups):
        cs_tiles = {}
        cur_eop = None
        for m in range(M_TILES):
            # narrow (512-wide) tiles come in PAIRS sharing one psum region
            # and ONE 1024-wide ACT instr (row sums split via 2 DVE reduces)
            paired = (kind == "x") or (kind == "s0" and TRI and m >= 4)
            if paired:
                if m % 2 == 1:
                    continue
                ps = pp.tile([P, PS_W], FP32, tag="super")
                for j in (0, 1):
                    mj = m + j
                    c0j = (MAIN + (0 if mj < 4 else 512)) if kind == "x" \
                        else 512
                    outj = ps[:, j * 512:(j + 1) * 512]
                    if fp8:
                        nc.tensor.matmul(outj,
                                         xm[:, :, mj * P:(mj + 1) * P],
                                         xm[:, :, c0j:c0j + 512],
                                         perf_mode=DR, start=True, stop=True)
                    else:
                        for k in (0, 1):
                            nc.tensor.matmul(
                                outj, xm[:, k, mj * P:(mj + 1) * P],
                                xm[:, k, c0j:c0j + 512],
                                start=(k == 0), stop=(k == 1))
                flush_pending()
                if pe_only:
                    continue
                if kind == "x":
                    cur_eop = eopp.tile([P, 2, 512], FP8 if fp8 else BF16,
                                        tag="eop512")
                    act_out = cur_eop[:, :, :]
                    slots = [cur_eop[:, 0, :], cur_eop[:, 1, :]]
                else:
                    eo = eo0p.tile([P, 1024], BF16, tag="eo0")
                    act_out = eo[:, 0:1024]
                    slots = [eo[:, 0:512], eo[:, 512:1024]]
                nc.scalar.activation(
                    out=act_out, in_=ps[:, 0:1024],
                    func=mybir.ActivationFunctionType.Exp, scale=ACT_SCALE)
                for j in (0, 1):
                    aidxj = gi * M_TILES + m + j
                    nc.vector.reduce_sum(acc_sb[:, aidxj:aidxj + 1],
                                         slots[j], axis=mybir.AxisListType.X)
                n_act += 2
                if kind == "x" and not no_cs:
                    chunk = 6 if m < 4 else 7
                    first, last = m in (0, 4), m in (2, 6)
                    if first:
                        cs_tiles[chunk] = ppc.tile([P, 512], FP32, tag="cs",
                                                   name=f"cs_{gi}_{chunk}")
                    cst = cs_tiles[chunk]
                    if fp8:
                        def mkp(cst=cst, rhs=cur_eop[:, :, 0:512],
                                first=first, last=last, chunk=chunk):
                            nc.tensor.matmul(
                                cst[0:1, :], ones_t[:, :, 0:1], rhs,
                                perf_mode=DR, start=first, stop=last)
                            if last:
                                nc.vector.tensor_copy(
                                    out=cs_sb[0:1, chunk * 512:
                                              (chunk + 1) * 512],
                                    in_=cst[0:1, :])
                        pending.append(mkp)
                    else:
                        for j in (0, 1):
                            def mkpb(cst=cst, rhs_b=slots[j],
                                     first=(first and j == 0),
                                     last=(last and j == 1), chunk=chunk):
                                nc.tensor.matmul(
                                    cst[0:1, :], ones_t[:, 0, 0:1], rhs_b,
                                    start=first, stop=last)
                                if last:
                                    nc.vector.tensor_copy(
                                        out=cs_sb[0:1, chunk * 512:
                                                  (chunk + 1) * 512],
                                        in_=cst[0:1, :])
                            pending.append(mkpb)
                continue
            c0, width = gc0, gw
            lhs = xm[:, :, m * P:(m + 1) * P]
            ps = pp.tile([P, PS_W], FP32, tag="super")
            for cc in range(width // 512):
                rhs = xm[:, :, c0 + cc * 512: c0 + (cc + 1) * 512]
                out = ps[:, cc * 512:(cc + 1) * 512]
                if fp8:
                    nc.tensor.matmul(out, lhs, rhs, perf_mode=DR,
                                     start=True, stop=True)
                else:
                    for k in (0, 1):
                        nc.tensor.matmul(out, xm[:, k, m * P:(m + 1) * P],
                                         xm[:, k, c0 + cc * 512:
                                            c0 + (cc + 1) * 512],
                                         start=(k == 0), stop=(k == 1))
            flush_pending()
            if pe_only:
                continue

            # exp target
            if kind == "s0":
                eo = eo0p.tile([P, 1024], BF16, tag="eo0")
                slot = eo[:, :width]
            else:
                if m % 2 == 0:
                    cur_eop = eopp.tile([P, 2, width],
                                        FP8 if fp8 else BF16,
                                        tag=f"eop{width}")
                slot = cur_eop[:, m % 2, :]

            aidx = gi * M_TILES + m
            if _use_accum(n_act):
                nc.scalar.activation(
                    out=slot, in_=ps[:, :width],
                    func=mybir.ActivationFunctionType.Exp, scale=ACT_SCALE,
                    accum_out=acc_sb[:, aidx:aidx + 1])
            else:
                nc.scalar.activation(
                    out=slot, in_=ps[:, :width],
                    func=mybir.ActivationFunctionType.Exp, scale=ACT_SCALE)
                nc.vector.reduce_sum(acc_sb[:, aidx:aidx + 1], slot,
                                     axis=mybir.AxisListType.X)
            n_act += 1

            # own-band triangle: cols [512,1024) x rows 0-511 also feed the
            # d of rows 512-1023 via colsum chunk 8 (bf16 stream, no DR —
            # eo0 stays bf16 because the diagonal holds e^10)
            if kind == "s0" and TRI and m < 4 and not no_cs:
                if m == 0:
                    cs_tiles[8] = ppc.tile([P, 512], FP32, tag="cs",
                                           name="cs_tri")
                cst8 = cs_tiles[8]
                rhs_t = slot[:, 512:1024]

                def mkt(cst8=cst8, rhs_t=rhs_t, first=(m == 0),
                        last=(m == 3)):
                    nc.tensor.matmul(cst8[0:1, :], ones_bf[:, 0:1], rhs_t,
                                     start=first, stop=last)
                    if last:
                        nc.vector.tensor_copy(
                            out=cs_sb[0:1, 8 * 512:9 * 512],
                            in_=cst8[0:1, :])
                pending.append(mkt)

            # colsum export for cols >= 1024 (delayed one iteration so the
            # in-order PE queue always has the next exp-mm first)
            if kind != "s0" and not no_cs:
                if fp8:
                    if m % 2 == 1:
                        for cc in range(width // 512):
                            if kind == "main":
                                chunk = (c0 + cc * 512 - 1024) // 512
                                first, last = (m == 1), (m == 7)
                            else:
                                chunk = 6 if m < 4 else 7
                                first = m in (1, 5)
                                last = m in (3, 7)
                            if first:
                                cs_tiles[chunk] = ppc.tile(
                                    [P, 512], FP32, tag="cs",
                                    name=f"cs_{gi}_{chunk}")
                            cst = cs_tiles[chunk]
                            rhs = cur_eop[:, :, cc * 512:(cc + 1) * 512]

                            def mk(cst=cst, rhs=rhs, first=first, last=last,
                                   chunk=chunk):
                                nc.tensor.matmul(
                                    cst[0:1, :], ones_t[:, :, 0:1], rhs,
                                    perf_mode=DR, start=first, stop=last)
                                if last:
                                    nc.vector.tensor_copy(
                                        out=cs_sb[0:1, chunk * 512:
                                                  (chunk + 1) * 512],
                                        in_=cst[0:1, :])
                            pending.append(mk)
                else:
                    for cc in range(width // 512):
                        if kind == "main":
                            chunk = (c0 + cc * 512 - 1024) // 512
                            first, last = (m == 0), (m == 7)
                        else:
                            chunk = 6 if m < 4 else 7
                            first = m in (0, 4)
                            last = m in (3, 7)
                        if first:
                            cs_tiles[chunk] = ppc.tile(
                                [P, 512], FP32, tag="cs",
                                name=f"cs_{gi}_{chunk}")
                        cst = cs_tiles[chunk]
                        rhs_b = slot[:, cc * 512:(cc + 1) * 512]

                        def mkb(cst=cst, rhs_b=rhs_b, first=first, last=last,
                                chunk=chunk):
                            nc.tensor.matmul(
                                cst[0:1, :], ones_t[:, 0, 0:1], rhs_b,
                                start=first, stop=last)
                            if last:
                                nc.vector.tensor_copy(
                                    out=cs_sb[0:1, chunk * 512:
                                              (chunk + 1) * 512],
                                    in_=cst[0:1, :])
                        pending.append(mkb)
    flush_pending(all_=True)


def _split_multi_waits(nc):
    """walrus codegen accepts at most ONE semaphore wait per engine
    instruction; hoist extra waits into standalone InstEventSemaphore ops."""
    n_split = 0
    for blk in nc.m.functions[0].blocks:
        new_insts = []
        for inst in blk.instructions:
            si = inst.sync_info
            tname = type(inst).__name__
            if si is not None and len(si.on_wait) > 1 and tname != "InstEventSemaphore":
                waits = list(si.on_wait)
                for j, w in enumerate(waits[:-1]):
                    es = mybir.InstEventSemaphore(
                        name=f"W-split-{inst.name}-{j}")
                    es.engine = inst.engine
                    es.sync_info = mybir.SyncInfo(on_wait=[w], on_update=[])
                    new_insts.append(es)
                    nc.register_instruction(es)
                    n_split += 1
                inst.sync_info = mybir.SyncInfo(
                    on_wait=[waits[-1]], on_update=list(si.on_update))
            new_insts.append(inst)
        blk.instructions[:] = new_insts
    return n_split


def _local_col_map(k):
    """Global column index for each of core k's 5120 local columns."""
    g = np.empty(LCOLS, dtype=np.int64)
    g[:MAIN] = (k * RPC + np.arange(MAIN)) % N
    if k < 4:
        b = k + 4
        g[4096:4608] = b * RPC + np.arange(512)
        g[4608:5120] = b * RPC + 512 + np.arange(512)
    else:
        a = k - 4
        g[4096:4608] = a * RPC + 512 + np.arange(512)
        g[4608:5120] = a * RPC + np.arange(512)
    return g


def _quantize(xn):
    import ml_dtypes
    if MM_DT == "fp8":
        q = (xn * SCALE_IN).astype(ml_dtypes.float8_e4m3)
    else:
        q = xn.astype(ml_dtypes.bfloat16)
    return q


def _prepare_inputs(x):
    import ml_dtypes
    x = np.ascontiguousarray(np.asarray(x, dtype=np.float32))
    inv = (1.0 / np.sqrt((x * x).sum(axis=1))).astype(np.float32)
    xn = x * inv[:, None]
    q = _quantize(xn)                       # [N, D] quantized
    edt = ml_dtypes.float8_e4m3 if MM_DT == "fp8" else ml_dtypes.bfloat16
    ones = np.ones((P, 2, 16), dtype=edt)
    in_maps = []
    for k in range(NCORES):
        g = _local_col_map(k)
        xmT = np.ascontiguousarray(q[g, :].T)          # [D, LCOLS]
        xm = np.ascontiguousarray(xmT.reshape(2, P, LCOLS))
        in_maps.append({"xm": xm, "ones": ones})
    return in_maps


def _host_diag_v(x):
    """Diag exp + pair logits recomputed from the quantized inputs (matches
    device arithmetic to fp32 rounding / ACT-spline ULPs)."""
    x = np.ascontiguousarray(np.asarray(x, dtype=np.float32))
    inv = (1.0 / np.sqrt((x * x).sum(axis=1))).astype(np.float32)
    xn = x * inv[:, None]
    xq = _quantize(xn).astype(np.float32)              # dequantized
    s_ii = (xq * xq).sum(axis=1, dtype=np.float32)
    diag_exp = np.exp(np.float64(s_ii * np.float32(ACT_SCALE)))
    i = np.arange(N)
    v_raw = (xq * xq[i ^ 1]).sum(axis=1, dtype=np.float32)
    v_over_tau = np.float64(v_raw) * ACT_SCALE
    return diag_exp, v_over_tau


def _combine(results, diag_exp, v_over_tau):
    rowsum = np.zeros(N, dtype=np.float64)
    colsum = np.zeros(N, dtype=np.float64)
    for k in range(NCORES):
        acc = np.asarray(results[k]["acc"], dtype=np.float64)  # [128, 40]
        cs = np.asarray(results[k]["cs"], dtype=np.float64).reshape(-1)
        g = _local_col_map(k)
        # rows of this core: i = k*1024 + m*128 + p ; acc col = s*8+m
        rs = acc.reshape(P, N_GROUPS, M_TILES).sum(axis=1)     # [p, m]
        idx = k * RPC + np.arange(M_TILES)[None, :] * P \
            + np.arange(P)[:, None]
        rowsum[idx.reshape(-1)] = rs.reshape(-1)
        # colsum chunks: j=0..5 -> local cols 1024+512j; j=6,7 -> 4096+...
        for j in range(N_CS):
            if j < 6:
                lc = 1024 + 512 * j
            elif j < 8:
                lc = 4096 + 512 * (j - 6)
            else:
                if not TRI:
                    continue
                lc = 512
            np.add.at(colsum, g[lc:lc + 512], cs[j * 512:(j + 1) * 512])
    d = rowsum + colsum + 1.0 - diag_exp
    loss = (np.log(d) - v_over_tau).sum() / N
    return np.float32(loss)


def kernel(x, repeat=None):
    if repeat is None:
        repeat = int(os.environ.get("KERNEL_REPEAT", "1"))
    key = f"nc{repeat}"
    if key not in _CACHE:
        _CACHE[key] = build_nc(repeat)
    nc = _CACHE[key]
    in_maps = _prepare_inputs(x)
    res = run_bass_kernel_spmd(nc, in_maps, list(range(NCORES)))
    _CACHE["last_results"] = res
    diag_exp, v_over_tau = _host_diag_v(x)
    return _combine(res.results, diag_exp, v_over_tau)
